# revision 1
# baseline (speedup 1.0000x reference)
"""GAT layer on 8 TRN2 NeuronCores (Bass/Tile).

Strategy (dst-partitioned, edge-parallel within core):
  - Core k owns dst nodes [k*12500, (k+1)*12500). Host sorts each core's
    edges by (src_chunk, dst) with src_chunk = src // 25000 (chunk-local src
    fits int16 for dma_gather), padding so every 128-edge tile's dst stays
    inside one aligned 128-node window.
  - Phase 0 (device): A = W_node-folded attn vectors ([64, 8]); a_pair[n] =
    x[n] @ A ([100K, 8]) written to a 256B-stride DRAM table; own-range
    a_dst rows copied to a local table indexed by core-local dst.
  - Main loop per 128-edge tile: dma_gather x[src] (256B rows), a_src[src]
    + a_dst[dst] (16B rows, 256B stride); e_score on PE (attrT tile
    stationary, W_edge moving); logit = e_score + a_src + a_dst;
    w = max(exp(logit-20), exp(0.2*logit-20))  (= exp(leakyrelu)-shifted;
    the shift cancels in the softmax ratio and keeps Reciprocal in range);
    indicator I[e,j] = (dstc[e]==j) via iota-compare; PSUM[128,68] +=
    I.T @ [x*w | w]: unnormalized per-node feature sums + weight sums.
  - Finalize per 128-node window: out = LN((agg/(sum+eps) + x_own) @ W_node).
"""
import sys

sys.path.insert(0, "/opt/trn_rl_repo")
import numpy as np
import ml_dtypes

import concourse.bass as bass
import concourse.mybir as mybir
import concourse.tile as tile
import concourse.bacc as bacc
from concourse.bass_utils import run_bass_kernel_spmd

FP32 = mybir.dt.float32
BF16 = mybir.dt.bfloat16
I16 = mybir.dt.int16
I32 = mybir.dt.int32
AF = mybir.ActivationFunctionType
OP = mybir.AluOpType
AX = mybir.AxisListType

N, E = 100000, 1600000
IN_DIM, OUT_DIM, EDGE_DIM, H = 64, 64, 16, 4
NEG = 0.2
EPS_SM = 1e-8
EPS_LN = 1e-5
NCORES = 8
NPC = N // NCORES            # 12500
CHUNK = 25000
NCHUNK = N // CHUNK          # 4
WIN = 128
NWIN = (NPC + WIN - 1) // WIN  # 98
GB = 8                       # tiles per gather batch (1024 edges; >1024 idx gathers fault)
LB = 8                       # tiles per logit batch
SHIFT = 0.0
LAST_EXEC_NS = None
_NC_CACHE = None


def dma_gather_small(gp, out_ap, in_ap, idxs_ap, num_idxs, elem_size, elem_step):
    """dma_gather with elem bytes not a 256-multiple (non-transpose).
    Row stride (elem_step elems) must still be a 256B multiple."""
    from concourse._compat import exact_div

    assert idxs_ap.dtype == mybir.dt.int16
    assert in_ap.ap[-1][1] == out_ap.ap[-1][1] == elem_size
    assert in_ap.ap[0][0] == elem_step
    stride_bytes_256 = exact_div(elem_step * mybir.dt.size(in_ap.dtype), 256)
    _in_ap = gp.lower_ap_dma(in_ap, for_custom_bir_dma=True)
    return gp.add_instruction(
        mybir.InstDMAGatherAnt(
            name=gp.bass.get_next_instruction_name(),
            ins=[*_in_ap, gp.lower_ap(idxs_ap), gp.lower_val_access(gp.to_reg(num_idxs))],
            outs=[gp.lower_ap(out_ap)],
            transpose=False, num_idxs=num_idxs, elem_size=elem_size,
            stride_bytes_256=stride_bytes_256, gen_mode=0, single_packet=True,
            queue_num=0, sbuf_tokens_per_rank=0, sbuf_free_dim_per_rank=0,
            sbuf_free_dim_pad_per_rank=0, sbuf_byte_offset=0,
        )
    )


def _host_prep(edge_index, edge_attr):
    src = np.asarray(edge_index[0], np.int64)
    dst = np.asarray(edge_index[1], np.int64)
    core = dst // NPC
    per_core = []
    for k in range(NCORES):
        m = np.nonzero(core == k)[0]
        dl = (dst[m] - k * NPC).astype(np.int32)
        ch = (src[m] // CHUNK).astype(np.int32)
        order = np.lexsort((dl, ch))
        m = m[order]; dl = dl[order]; ch = ch[order]
        sl = (src[m] - ch * CHUNK).astype(np.int32)
        per_core.append((m, dl, ch, sl, dl // WIN))

    # shared tiles-per-(chunk, window): max over cores
    T = np.zeros((NCHUNK, NWIN), np.int64)
    for k in range(NCORES):
        m, dl, ch, sl, w = per_core[k]
        cnt = np.zeros((NCHUNK, NWIN), np.int64)
        np.add.at(cnt, (ch, w), 1)
        T = np.maximum(T, (cnt + 127) // 128)
    # pad each chunk's tile total to a GB multiple (extra tiles join last window)
    for c in range(NCHUNK):
        T[c, NWIN - 1] += (-int(T[c].sum())) % GB
    T = T.astype(int)
    tiles_per_chunk = [int(T[c].sum()) for c in range(NCHUNK)]
    Ttot = int(T.sum())
    Etot = Ttot * 128

    attr = np.asarray(edge_attr, np.float32)
    streams = []
    for k in range(NCORES):
        m, dl, ch, sl, w = per_core[k]
        gsl = np.zeros(Etot, np.int32)
        gdl = np.zeros(Etot, np.int32)
        dstc = np.full(Etot, -1.0, np.float32)
        eid = np.full(Etot, -1, np.int64)
        cnt = np.zeros((NCHUNK, NWIN), np.int64)
        np.add.at(cnt, (ch, w), 1)
        ptr = 0; pos = 0
        for c in range(NCHUNK):
            for ww in range(NWIN):
                n_here = int(cnt[c, ww])
                if n_here:
                    gsl[pos:pos + n_here] = sl[ptr:ptr + n_here]
                    gdl[pos:pos + n_here] = dl[ptr:ptr + n_here]
                    dstc[pos:pos + n_here] = (dl[ptr:ptr + n_here] - ww * WIN).astype(np.float32)
                    eid[pos:pos + n_here] = m[ptr:ptr + n_here]
                ptr += n_here
                pos += int(T[c, ww]) * 128
        assert ptr == len(dl) and pos == Etot

        def wrap16(vals):
            # per 4096-edge batch: local position i -> [16g + i%16, b*256 + i//16]
            nb = Etot // (GB * 128)
            v = vals.reshape(nb, GB * 128)
            cols = GB * 128 // 16
            out = np.zeros((128, nb * cols), np.int16)
            blk = v.reshape(nb, cols, 16).transpose(0, 2, 1).astype(np.int16)  # [nb,16,cols]
            for g in range(8):
                out[16 * g:16 * (g + 1), :] = np.concatenate(list(blk), axis=1)
            return out

        at = np.zeros((16, Etot), ml_dtypes.bfloat16)
        valid = eid >= 0
        at[:, valid] = attr[eid[valid]].T.astype(ml_dtypes.bfloat16)
        streams.append(dict(
            gidx=wrap16(gsl), didx=wrap16(gdl),
            dstc=np.ascontiguousarray(dstc.reshape(-1, 128).T), attrT=at))
    return streams, T, tiles_per_chunk, Etot


def _build_program(T, tiles_per_chunk, Etot):
    nc = bacc.Bacc(None, target_bir_lowering=False, debug=False)
    Ttot = int(T.sum())

    t_x = nc.declare_dram_parameter("x", [N, IN_DIM], FP32, isOutput=False)
    t_xT = nc.declare_dram_parameter("xT_bf", [IN_DIM, N], BF16, isOutput=False)
    t_WnT = nc.declare_dram_parameter("WnT", [OUT_DIM, IN_DIM], FP32, isOutput=False)
    t_asT = nc.declare_dram_parameter("attn_srcT", [16, H], FP32, isOutput=False)
    t_adT = nc.declare_dram_parameter("attn_dstT", [16, H], FP32, isOutput=False)
    t_We = nc.declare_dram_parameter("W_edge", [EDGE_DIM, H], FP32, isOutput=False)
    t_Wn = nc.declare_dram_parameter("W_node", [IN_DIM, OUT_DIM], FP32, isOutput=False)
    t_gam = nc.declare_dram_parameter("gamma_row", [1, OUT_DIM], FP32, isOutput=False)
    t_bet = nc.declare_dram_parameter("beta_row", [1, OUT_DIM], FP32, isOutput=False)
    t_iota = nc.declare_dram_parameter("iota_bf", [128, 128], BF16, isOutput=False)
    t_iden = nc.declare_dram_parameter("iden", [128, 128], FP32, isOutput=False)
    t_ones = nc.declare_dram_parameter("ones_col", [1, 128], FP32, isOutput=False)
    t_cid = nc.declare_dram_parameter("coreid", [1, 1], I32, isOutput=False)
    t_xTo = nc.declare_dram_parameter("xT_own", [IN_DIM, NPC], BF16, isOutput=False)
    t_gidx = nc.declare_dram_parameter("gidx", [128, Etot // 16], I16, isOutput=False)
    t_didx = nc.declare_dram_parameter("didx", [128, Etot // 16], I16, isOutput=False)
    t_dstc = nc.declare_dram_parameter("dstc", [128, Ttot], FP32, isOutput=False)
    t_attrT = nc.declare_dram_parameter("attrT", [16, Etot], BF16, isOutput=False)
    t_out = nc.declare_dram_parameter("out", [NPC, OUT_DIM], FP32, isOutput=True)

    t_apair = nc.dram_tensor("apair", [N, 64], FP32)      # 256B rows; cols 0:4 = a_src
    t_htab = nc.dram_tensor("htab", [N, 64], FP32)        # h = x @ W_node
    t_hown = nc.dram_tensor("hown", [NPC, 64], FP32)      # own h rows (residual)
    t_adst = nc.dram_tensor("adst_tab", [NPC, 64], FP32)  # local a_dst; cols 0:4

    with tile.TileContext(nc) as tc, tc.tile_pool(name="const", bufs=1) as cpool:
        with (
            tc.tile_pool(name="ph0", bufs=3) as p0,
            tc.tile_pool(name="ph0ps", bufs=2, space="PSUM") as p0ps,
        ):
            # scalar constants for ACT bias/scale (per-partition columns)
            sb_k = cpool.tile([128, 5], FP32, tag="konst")
            nc.vector.memset(sb_k[:, 0:1], -SHIFT)
            nc.vector.memset(sb_k[:, 1:2], NEG)
            nc.vector.memset(sb_k[:, 2:3], EPS_SM)
            nc.vector.memset(sb_k[:, 3:4], 1.0 / 64.0)
            nc.vector.memset(sb_k[:, 4:5], EPS_LN)
            k_shift = sb_k[:, 0:1]; k_neg = sb_k[:, 1:2]; k_epssm = sb_k[:, 2:3]
            k_inv64 = sb_k[:, 3:4]; k_epsln = sb_k[:, 4:5]

            sb_iota = cpool.tile([128, 128], BF16, tag="iota")
            nc.sync.dma_start(out=sb_iota[:], in_=t_iota[:])
            sb_iden = cpool.tile([128, 128], FP32, tag="iden")
            nc.sync.dma_start(out=sb_iden[:], in_=t_iden[:])
            sb_We_f = cpool.tile([EDGE_DIM, H], FP32, tag="Wef")
            nc.sync.dma_start(out=sb_We_f[:], in_=t_We[:])
            sb_We = cpool.tile([EDGE_DIM, H], BF16, tag="We")
            nc.vector.tensor_copy(out=sb_We[:], in_=sb_We_f[:])
            sb_Wn = cpool.tile([IN_DIM, OUT_DIM], FP32, tag="Wn")
            nc.sync.dma_start(out=sb_Wn[:], in_=t_Wn[:])
            sb_Wn_bf = cpool.tile([IN_DIM, OUT_DIM], BF16, tag="Wnbf")
            nc.vector.tensor_copy(out=sb_Wn_bf[:], in_=sb_Wn[:])
            sb_ones = cpool.tile([1, 128], FP32, tag="ones")
            nc.sync.dma_start(out=sb_ones[:], in_=t_ones[:])
            sb_cid = cpool.tile([1, 1], I32, tag="cid")
            nc.sync.dma_start(out=sb_cid[:], in_=t_cid[:])

            sb_gr = cpool.tile([1, OUT_DIM], FP32, tag="gr")
            nc.sync.dma_start(out=sb_gr[:], in_=t_gam[:])
            sb_br = cpool.tile([1, OUT_DIM], FP32, tag="br")
            nc.sync.dma_start(out=sb_br[:], in_=t_bet[:])
            ps_gb = p0ps.tile([128, 2 * OUT_DIM], FP32, tag="gb")
            nc.tensor.matmul(ps_gb[:, 0:OUT_DIM], sb_ones[:], sb_gr[:], start=True, stop=False)
            nc.tensor.matmul(ps_gb[:, OUT_DIM:], sb_ones[:], sb_br[:], start=False, stop=True)
            sb_gam = cpool.tile([128, OUT_DIM], FP32, tag="gamt")
            nc.vector.tensor_copy(out=sb_gam[:], in_=ps_gb[:, 0:OUT_DIM])
            sb_bet = cpool.tile([128, OUT_DIM], FP32, tag="bett")
            nc.vector.tensor_copy(out=sb_bet[:], in_=ps_gb[:, OUT_DIM:])

            # A = [A_src | A_dst] ([64, 8]):  A_src[:,h] = WnT[16h:16h+16,:].T @ attn_srcT[16h:16h+16,h]
            # per-head 16-row blocks of WnT packed at partition 0
            sb_WnT = cpool.tile([16, H * IN_DIM], FP32, tag="WnT")
            for h in range(H):
                nc.sync.dma_start(out=sb_WnT[:, h * IN_DIM:(h + 1) * IN_DIM],
                                  in_=t_WnT[16 * h:16 * (h + 1), :])
            sb_asT = cpool.tile([16, H], FP32, tag="asT")
            nc.sync.dma_start(out=sb_asT[:], in_=t_asT[:])
            sb_adT = cpool.tile([16, H], FP32, tag="adT")
            nc.sync.dma_start(out=sb_adT[:], in_=t_adT[:])
            ps_A = p0ps.tile([IN_DIM, 2 * H], FP32, tag="Acomb")
            for h in range(H):
                nc.tensor.matmul(ps_A[:, h:h + 1], sb_WnT[:, h * IN_DIM:(h + 1) * IN_DIM],
                                 sb_asT[:, h:h + 1], start=True, stop=True)
                nc.tensor.matmul(ps_A[:, H + h:H + h + 1], sb_WnT[:, h * IN_DIM:(h + 1) * IN_DIM],
                                 sb_adT[:, h:h + 1], start=True, stop=True)
            sb_A = cpool.tile([IN_DIM, 2 * H], BF16, tag="Abf")
            nc.vector.tensor_copy(out=sb_A[:], in_=ps_A[:])

            # phase 0: apair[n, 0:8] = x[n] @ A ; htab[n] = x[n] @ W_node
            nslices = (N + 127) // 128
            for s in range(nslices):
                n0 = s * 128
                nn = min(128, N - n0)
                lhs = p0.tile([IN_DIM, 128], BF16, tag="xTs")
                nc.sync.dma_start(out=lhs[:, 0:nn], in_=t_xT[:, n0:n0 + nn])
                ps = p0ps.tile([128, 2 * H], FP32, tag="aps")
                nc.tensor.matmul(ps[:], lhs[:], sb_A[:], start=True, stop=True)
                stg = p0.tile([128, 2 * H], FP32, tag="astg")
                nc.vector.tensor_copy(out=stg[:], in_=ps[:])
                nc.sync.dma_start(out=t_apair[n0:n0 + nn, 0:8], in_=stg[0:nn, :])
                psh = p0ps.tile([128, OUT_DIM], FP32, tag="hps")
                nc.tensor.matmul(psh[:], lhs[:], sb_Wn_bf[:], start=True, stop=True)
                stgh = p0.tile([128, OUT_DIM], FP32, tag="hstg")
                nc.vector.tensor_copy(out=stgh[:], in_=psh[:])
                nc.sync.dma_start(out=t_htab[n0:n0 + nn, :], in_=stgh[0:nn, :])
            # phase 0b: own-range a_dst + h from the per-core xT_own input
            nslo = (NPC + 127) // 128
            for s in range(nslo):
                n0 = s * 128
                nn = min(128, NPC - n0)
                lhs = p0.tile([IN_DIM, 128], BF16, tag="xTs")
                nc.sync.dma_start(out=lhs[:, 0:nn], in_=t_xTo[:, n0:n0 + nn])
                ps = p0ps.tile([128, 2 * H], FP32, tag="aps")
                nc.tensor.matmul(ps[:], lhs[:], sb_A[:], start=True, stop=True)
                stg = p0.tile([128, 2 * H], FP32, tag="astg")
                nc.vector.tensor_copy(out=stg[:], in_=ps[:])
                nc.sync.dma_start(out=t_adst[n0:n0 + nn, 0:4], in_=stg[0:nn, 4:8])
                psh = p0ps.tile([128, OUT_DIM], FP32, tag="hps")
                nc.tensor.matmul(psh[:], lhs[:], sb_Wn_bf[:], start=True, stop=True)
                stgh = p0.tile([128, OUT_DIM], FP32, tag="hstg")
                nc.vector.tensor_copy(out=stgh[:], in_=psh[:])
                nc.sync.dma_start(out=t_hown[n0:n0 + nn, :], in_=stgh[0:nn, :])

        # ---------------- main edge loop ----------------
        with (
            tc.tile_pool(name="mn", bufs=2) as mp,
            tc.tile_pool(name="mn3", bufs=3) as mp3,
            tc.tile_pool(name="accp", bufs=1) as ap_,
            tc.tile_pool(name="lpsp", bufs=2, space="PSUM") as lpsp,
            tc.tile_pool(name="apsp", bufs=2, space="PSUM") as apsp,
        ):
            acc = ap_.tile([128, NWIN * 68], FP32, tag="acc")
            nc.vector.memset(acc[:], 0)

            # (window, pos-in-run, run-len) per tile, per chunk
            t_glob = 0
            for c in range(NCHUNK):
                run_tiles = []
                for wdx in range(NWIN):
                    for i in range(int(T[c, wdx])):
                        run_tiles.append((wdx, i, int(T[c, wdx])))
                tcn = tiles_per_chunk[c]
                assert tcn % GB == 0 and len(run_tiles) == tcn
                agg_ps = None
                for b in range(tcn // GB):
                    tb0 = t_glob + b * GB
                    # --- gather batch ---
                    xrows = mp.tile([128, GB, 64], FP32, tag="xrows")
                    asr = mp.tile([128, GB, 4], FP32, tag="asr")
                    adr = mp.tile([128, GB, 4], FP32, tag="adr")
                    attrs = mp.tile([16, GB * 128], BF16, tag="attrs")
                    dstcs = mp.tile([128, GB], FP32, tag="dstcs")
                    gidx_sb = mp.tile([128, GB * 8], I16, tag="gidx")
                    didx_sb = mp.tile([128, GB * 8], I16, tag="didx")
                    i0 = tb0 * 8
                    e0 = tb0 * 128
                    nc.sync.dma_start(out=attrs[:], in_=t_attrT[:, e0:e0 + GB * 128])
                    nc.sync.dma_start(out=dstcs[:], in_=t_dstc[:, tb0:tb0 + GB])
                    nc.sync.dma_start(out=gidx_sb[:], in_=t_gidx[:, i0:i0 + GB * 8])
                    nc.sync.dma_start(out=didx_sb[:], in_=t_didx[:, i0:i0 + GB * 8])
                    nc.gpsimd.dma_gather(
                        xrows[:], t_htab[c * CHUNK:(c + 1) * CHUNK, :], gidx_sb[:],
                        GB * 128, GB * 128, 64)
                    dma_gather_small(
                        nc.gpsimd, asr[:], t_apair[c * CHUNK:(c + 1) * CHUNK, 0:4],
                        gidx_sb[:], GB * 128, 4, 64)
                    dma_gather_small(
                        nc.gpsimd, adr[:], t_adst[:, 0:4], didx_sb[:],
                        GB * 128, 4, 64)
                    for lb0 in range(0, GB, LB):
                        # --- logit batch: e_score MMs, adds, exp ---
                        logit_ps = lpsp.tile([128, LB * 4], FP32, tag="lps")
                        for j in range(LB):
                            bt = lb0 + j
                            nc.tensor.matmul(
                                logit_ps[:, 4 * j:4 * j + 4],
                                attrs[:, bt * 128:(bt + 1) * 128], sb_We[:],
                                start=True, stop=True)
                        lg = mp3.tile([128, LB * 4], FP32, tag="lg")
                        nc.vector.tensor_tensor(
                            out=lg[:], in0=logit_ps[:],
                            in1=asr[:, lb0:lb0 + LB, :].rearrange("p a b -> p (a b)"),
                            op=OP.add)
                        nc.vector.tensor_tensor(
                            out=lg[:], in0=lg[:],
                            in1=adr[:, lb0:lb0 + LB, :].rearrange("p a b -> p (a b)"),
                            op=OP.add)
                        e1 = mp3.tile([128, LB * 4], FP32, tag="e1")
                        nc.scalar.activation(out=e1[:], in_=lg[:], func=AF.Exp, bias=k_shift)
                        wexp = mp3.tile([128, LB * 4], FP32, tag="wexp")
                        nc.scalar.activation(out=wexp[:], in_=lg[:], func=AF.Exp,
                                             scale=k_neg, bias=k_shift)
                        nc.vector.tensor_tensor(out=wexp[:], in0=wexp[:], in1=e1[:], op=OP.max)
                        # --- per-tile: indicator, msg, aggregation ---
                        for j in range(LB):
                            bt = lb0 + j
                            tt = b * GB + bt
                            wdx, i_run, rlen = run_tiles[tt]
                            ind = mp3.tile([128, 128], BF16, tag="ind")
                            nc.vector.tensor_scalar(
                                out=ind[:], in0=sb_iota[:], scalar1=dstcs[:, bt:bt + 1],
                                scalar2=None, op0=OP.is_equal)
                            msg = mp3.tile([128, 68], BF16, tag="msg")
                            w_b = wexp[:, 4 * j:4 * j + 4].unsqueeze(2).broadcast_to([128, 4, 16])
                            nc.vector.tensor_tensor(
                                out=msg[:, 0:64].rearrange("p (a b) -> p a b", a=4),
                                in0=xrows[:, bt, :].rearrange("p (a b) -> p a b", a=4),
                                in1=w_b, op=OP.mult)
                            nc.vector.tensor_copy(out=msg[:, 64:68], in_=wexp[:, 4 * j:4 * j + 4])
                            if i_run == 0:
                                agg_ps = apsp.tile([128, 68], FP32, tag="aggps")
                            nc.tensor.matmul(agg_ps[:], ind[:], msg[:],
                                             start=(i_run == 0), stop=(i_run == rlen - 1))
                            if i_run == rlen - 1:
                                nc.vector.tensor_tensor(
                                    out=acc[:, wdx * 68:(wdx + 1) * 68],
                                    in0=acc[:, wdx * 68:(wdx + 1) * 68],
                                    in1=agg_ps[:], op=OP.add)
                t_glob += tcn

            # ---------------- finalize ----------------
            with (
                tc.tile_pool(name="fin", bufs=3) as fp,
                tc.tile_pool(name="finps", bufs=2, space="PSUM") as fps,
            ):
                for wdx in range(NWIN):
                    n0 = wdx * 128
                    nn = min(128, NPC - n0)
                    xo = fp.tile([128, 64], FP32, tag="xo")
                    nc.sync.dma_start(out=xo[0:nn, :], in_=t_hown[n0:n0 + nn, :])
                    rcp = fp.tile([128, 4], FP32, tag="rcp")
                    nc.vector.tensor_scalar(out=rcp[:], in0=acc[:, wdx * 68 + 64:wdx * 68 + 68],
                                            scalar1=k_epssm, scalar2=None, op0=OP.add)
                    nc.vector.reciprocal(out=rcp[:], in_=rcp[:])
                    y = fp.tile([128, 64], FP32, tag="y")
                    r_b = rcp[:].unsqueeze(2).broadcast_to([128, 4, 16])
                    nc.vector.tensor_tensor(
                        out=y[:].rearrange("p (a b) -> p a b", a=4),
                        in0=acc[:, wdx * 68:wdx * 68 + 64].rearrange("p (a b) -> p a b", a=4),
                        in1=r_b, op=OP.mult)
                    nc.vector.tensor_tensor(out=y[0:nn, :], in0=y[0:nn, :],
                                            in1=xo[0:nn, :], op=OP.add)
                    mu = fp.tile([128, 1], FP32, tag="mu")
                    nc.vector.tensor_reduce(out=mu[:], in_=y[:], axis=AX.X, op=OP.add)
                    mus = fp.tile([128, 1], FP32, tag="mus")
                    nc.vector.tensor_scalar(out=mus[:], in0=mu[:], scalar1=1.0 / 64.0,
                                            scalar2=None, op0=OP.mult)
                    zc = fp.tile([128, 64], FP32, tag="zc")
                    nc.vector.tensor_scalar(out=zc[:], in0=y[:], scalar1=mus[:],
                                            scalar2=None, op0=OP.subtract)
                    sq = fp.tile([128, 64], FP32, tag="sq")
                    vs = fp.tile([128, 1], FP32, tag="vs")
                    nc.scalar.activation(out=sq[:], in_=zc[:], func=AF.Square, accum_out=vs[:])
                    rstd = fp.tile([128, 1], FP32, tag="rstd")
                    nc.vector.tensor_scalar(out=rstd[:], in0=vs[:], scalar1=k_inv64,
                                            scalar2=k_epsln, op0=OP.mult, op1=OP.add)
                    nc.scalar.activation(out=rstd[:], in_=rstd[:], func=AF.Sqrt)
                    nc.vector.reciprocal(out=rstd[:], in_=rstd[:])
                    o = fp.tile([128, 64], FP32, tag="o")
                    nc.vector.tensor_scalar(out=o[:], in0=zc[:], scalar1=rstd[:],
                                            scalar2=None, op0=OP.mult)
                    nc.vector.tensor_tensor(out=o[:], in0=o[:], in1=sb_gam[:], op=OP.mult)
                    nc.vector.tensor_tensor(out=o[:], in0=o[:], in1=sb_bet[:], op=OP.add)
                    nc.sync.dma_start(out=t_out[n0:n0 + nn, :], in_=o[0:nn, :])

    nc.compile()
    return nc


def kernel(x, edge_index, edge_attr, W_node, W_edge, attn_src, attn_dst, ln_gamma, ln_beta):
    x = np.asarray(x, np.float32)
    W_node = np.asarray(W_node, np.float32)
    W_edge = np.asarray(W_edge, np.float32)
    attn_src = np.asarray(attn_src, np.float32)
    attn_dst = np.asarray(attn_dst, np.float32)
    ln_gamma = np.asarray(ln_gamma, np.float32)
    ln_beta = np.asarray(ln_beta, np.float32)

    streams, T, tiles_per_chunk, Etot = _host_prep(edge_index, edge_attr)
    global _NC_CACHE
    key = (T.tobytes(), Etot)
    if _NC_CACHE is not None and _NC_CACHE[0] == key:
        nc = _NC_CACHE[1]
    else:
        nc = _build_program(T, tiles_per_chunk, Etot)
        _NC_CACHE = (key, nc)

    shared = dict(
        x=x,
        xT_bf=np.ascontiguousarray(x.T).astype(ml_dtypes.bfloat16),
        WnT=np.ascontiguousarray(W_node.T),
        attn_srcT=np.ascontiguousarray(attn_src.T),
        attn_dstT=np.ascontiguousarray(attn_dst.T),
        W_edge=W_edge, W_node=W_node,
        gamma_row=np.ascontiguousarray(ln_gamma[None, :]),
        beta_row=np.ascontiguousarray(ln_beta[None, :]),
        iota_bf=np.tile(np.arange(128, dtype=np.float32)[None, :], (128, 1)).astype(ml_dtypes.bfloat16),
        iden=np.eye(128, dtype=np.float32),
        ones_col=np.ones((1, 128), np.float32),
    )
    in_maps = []
    for k in range(NCORES):
        m = dict(shared)
        m["coreid"] = np.array([[k]], np.int32)
        m["xT_own"] = np.ascontiguousarray(shared["xT_bf"][:, k * NPC:(k + 1) * NPC])
        for key in ("gidx", "didx", "dstc", "attrT"):
            m[key] = np.asarray(streams[k][key])
        in_maps.append(m)

    import time as _time
    _t0 = _time.time()
    res = run_bass_kernel_spmd(nc, in_maps, list(range(NCORES)))
    global LAST_EXEC_NS
    LAST_EXEC_NS = getattr(res, "exec_time_ns", None)
    if LAST_EXEC_NS is None:
        LAST_EXEC_NS = int((_time.time() - _t0) * 1e9)
    return np.concatenate([res.results[k]["out"] for k in range(NCORES)], 0).astype(np.float32)



# revision 2
# speedup vs baseline: 5555.9194x; 5555.9194x over previous
"""GAT layer on 8 TRN2 NeuronCores (Bass/Tile) — v2.

Strategy (dst-partitioned, edge-parallel within core), per core:
  - Core k owns dst nodes [k*12500, (k+1)*12500). Host groups its edges by
    (src_chunk, dst_window) cells, padding each cell to 128-edge tiles with a
    tile schedule shared across cores (max over cores), so the SPMD program
    is identical everywhere.
  - Phase 0: tab_c[n] = [h(n) bf16 x64 | a_src(n) bf16 x4] for all N nodes
    (4 chunk tables of 25088 rows, 256B stride); own-range h (f32) and a_dst
    (bf16) stay SBUF-resident.
  - Main loop per 1024-edge batch (8 tiles): ONE dma_gather (136B rows);
    indicator built for all 8 tiles in one DVE op; indicator transposed on
    PE (identity matmul) to add a_dst via matmul into the logit PSUM along
    with e_score; w = max(exp(lg), exp(0.2*lg)); agg PSUM accumulates
    ind^T @ [h*w | w] per (chunk, window) run.
  - Finalize fused at each window's last run: LN((agg/(sum+eps) + h_own)).
"""
import sys

sys.path.insert(0, "/opt/trn_rl_repo")
import numpy as np
import ml_dtypes

import concourse.bass as bass
import concourse.mybir as mybir
import concourse.tile as tile
import concourse.bacc as bacc

FP32 = mybir.dt.float32
BF16 = mybir.dt.bfloat16
I16 = mybir.dt.int16
AF = mybir.ActivationFunctionType
OP = mybir.AluOpType
AX = mybir.AxisListType

N, E = 100000, 1600000
IN_DIM, OUT_DIM, EDGE_DIM, H = 64, 64, 16, 4
NEG = 0.2
EPS_SM = 1e-8
EPS_LN = 1e-5
NCORES = 8
NPC = N // NCORES            # 12500
CHUNK = 25000
CHUNKP = 25088               # padded to 512 multiple
NCHUNK = N // CHUNK          # 4
WIN = 128
NWIN = (NPC + WIN - 1) // WIN  # 98
GB = 8                       # tiles per gather batch (1024 idx max per gather)
LAST_EXEC_NS = None
_CACHE = {}
_LAST_STAGED = None


def dma_gather_small(gp, out_ap, in_ap, idxs_ap, num_idxs, elem_size, elem_step):
    """dma_gather with elem bytes not a 256-multiple (non-transpose).
    Row stride (elem_step elems) must still be a 256B multiple."""
    from concourse._compat import exact_div

    assert idxs_ap.dtype == mybir.dt.int16
    assert in_ap.ap[-1][1] == out_ap.ap[-1][1] == elem_size
    assert in_ap.ap[0][0] == elem_step
    stride_bytes_256 = exact_div(elem_step * mybir.dt.size(in_ap.dtype), 256)
    _in_ap = gp.lower_ap_dma(in_ap, for_custom_bir_dma=True)
    return gp.add_instruction(
        mybir.InstDMAGatherAnt(
            name=gp.bass.get_next_instruction_name(),
            ins=[*_in_ap, gp.lower_ap(idxs_ap), gp.lower_val_access(gp.to_reg(num_idxs))],
            outs=[gp.lower_ap(out_ap)],
            transpose=False, num_idxs=num_idxs, elem_size=elem_size,
            stride_bytes_256=stride_bytes_256, gen_mode=0, single_packet=True,
            queue_num=0, sbuf_tokens_per_rank=0, sbuf_free_dim_per_rank=0,
            sbuf_free_dim_pad_per_rank=0, sbuf_byte_offset=0,
        )
    )


def _host_prep(edge_index, edge_attr):
    src = np.asarray(edge_index[0]).astype(np.int64).astype(np.int32)
    dst = np.asarray(edge_index[1]).astype(np.int64).astype(np.int32)
    core = dst // NPC
    ch = src // CHUNK
    dl = dst - core * NPC
    w = dl >> 7
    ncell = NCORES * NCHUNK * NWIN
    cell = (core * NCHUNK + ch) * NWIN + w
    cnt = np.bincount(cell, minlength=ncell).reshape(NCORES, NCHUNK, NWIN)
    T = ((cnt.max(0) + 127) // 128).astype(np.int64)      # [NCHUNK, NWIN]
    for c in range(NCHUNK):
        T[c, NWIN - 1] += (-int(T[c].sum())) % GB
    Ttot = int(T.sum())
    Etot = Ttot * 128
    cap = T * 128
    base_flat = np.concatenate([[0], np.cumsum(cap.flatten())[:-1]])
    base = base_flat.reshape(NCHUNK, NWIN)

    order = np.argsort(cell, kind="stable")
    cell_s = cell[order]
    cnt_flat = np.bincount(cell, minlength=ncell)
    starts = np.concatenate([[0], np.cumsum(cnt_flat)[:-1]])
    rank = np.arange(E, dtype=np.int64) - starts[cell_s]
    ch_s = ch[order]
    w_s = w[order]
    core_s = core[order]
    pos = base[ch_s, w_s] + rank
    sl = (src[order] - ch_s * CHUNK).astype(np.int16)
    dstc_val = (dl[order] - (w_s << 7)).astype(np.float32)
    attr_bf = np.asarray(edge_attr, np.float32).astype(ml_dtypes.bfloat16)[order]

    NB = Ttot // GB
    streams = []
    for k in range(NCORES):
        m = core_s == k
        pk = pos[m]
        gsl = np.zeros(Etot, np.int16)
        gsl[pk] = sl[m]
        dstc = np.full(Etot, -1.0, dtype=ml_dtypes.bfloat16)
        dstc[pk] = dstc_val[m].astype(ml_dtypes.bfloat16)
        at = np.zeros((Etot, EDGE_DIM), ml_dtypes.bfloat16)
        at[pk] = attr_bf[m]
        g3 = gsl.reshape(NB, 64, 16).transpose(0, 2, 1)               # [NB,16,64]
        g128 = np.broadcast_to(g3[:, None], (NB, 8, 16, 64)).reshape(NB, 128, 64)
        d128 = dstc.reshape(NB, 8, 128).transpose(0, 2, 1)            # [NB,128,8]
        meta = np.concatenate([g128, d128.view(np.int16)], axis=2)    # [NB,128,72]
        meta = np.ascontiguousarray(meta.transpose(1, 0, 2)).reshape(128, NB * 72)
        attrT = np.ascontiguousarray(at.T)                            # [16, Etot]
        streams.append(dict(meta=meta, attrT=attrT))
    return streams, T, Etot


def _build_program(T, Etot, reps):
    nc = bacc.Bacc(None, target_bir_lowering=False, debug=False)
    Ttot = int(T.sum())
    NB = Ttot // GB
    tiles_chunk = [int(T[c].sum()) for c in range(NCHUNK)]
    # last chunk having tiles for each window (finalize trigger)
    last_c = [max(c for c in range(NCHUNK) if T[c, w] > 0) for w in range(NWIN)]

    t_xT = nc.declare_dram_parameter("xT_bf", [IN_DIM, N], BF16, isOutput=False)
    t_xTo = nc.declare_dram_parameter("xT_own", [IN_DIM, NPC], BF16, isOutput=False)
    t_WnT = nc.declare_dram_parameter("WnT", [OUT_DIM, IN_DIM], FP32, isOutput=False)
    t_asT = nc.declare_dram_parameter("attn_srcT", [16, H], FP32, isOutput=False)
    t_adT = nc.declare_dram_parameter("attn_dstT", [16, H], FP32, isOutput=False)
    t_We = nc.declare_dram_parameter("W_edge", [EDGE_DIM, H], FP32, isOutput=False)
    t_Wn = nc.declare_dram_parameter("W_node", [IN_DIM, OUT_DIM], FP32, isOutput=False)
    t_gam = nc.declare_dram_parameter("gamma_row", [1, OUT_DIM], FP32, isOutput=False)
    t_bet = nc.declare_dram_parameter("beta_row", [1, OUT_DIM], FP32, isOutput=False)
    t_iota = nc.declare_dram_parameter("iota_bf", [128, 128], BF16, isOutput=False)
    t_iden = nc.declare_dram_parameter("iden_bf", [128, 128], BF16, isOutput=False)
    t_ones = nc.declare_dram_parameter("ones_col", [1, 128], FP32, isOutput=False)
    t_meta = nc.declare_dram_parameter("meta", [128, NB * 72], I16, isOutput=False)
    t_attrT = nc.declare_dram_parameter("attrT", [16, Etot], BF16, isOutput=False)
    t_out = nc.declare_dram_parameter("out", [NPC, OUT_DIM], FP32, isOutput=True)

    t_tabs = [nc.dram_tensor(f"tab{c}", [CHUNKP, 128], I16) for c in range(NCHUNK)]

    with tile.TileContext(nc) as tc, tc.tile_pool(name="const", bufs=1) as cpool:
        # ---- constants ----
        sb_k = cpool.tile([128, 3], FP32, tag="konst")
        nc.vector.memset(sb_k[:, 0:1], NEG)
        nc.vector.memset(sb_k[:, 1:2], EPS_SM)
        nc.vector.memset(sb_k[:, 2:3], 1.0 / 64.0)
        k_neg = sb_k[:, 0:1]
        k_epssm = sb_k[:, 1:2]
        k_inv64 = sb_k[:, 2:3]

        sb_iota = cpool.tile([128, 128], BF16, tag="iota")
        nc.sync.dma_start(out=sb_iota[:], in_=t_iota[:])
        sb_iden = cpool.tile([128, 128], BF16, tag="iden")
        nc.sync.dma_start(out=sb_iden[:], in_=t_iden[:])
        sb_We_f = cpool.tile([EDGE_DIM, H], FP32, tag="Wef")
        nc.sync.dma_start(out=sb_We_f[:], in_=t_We[:])
        sb_We = cpool.tile([EDGE_DIM, H], BF16, tag="We")
        nc.vector.tensor_copy(out=sb_We[:], in_=sb_We_f[:])
        sb_We8 = cpool.tile([EDGE_DIM, 2 * H], BF16, tag="We8")
        nc.vector.memset(sb_We8[:], 0)
        nc.vector.tensor_copy(out=sb_We8[:, 0:H], in_=sb_We_f[:])
        sb_Wn = cpool.tile([IN_DIM, OUT_DIM], FP32, tag="Wn")
        nc.sync.dma_start(out=sb_Wn[:], in_=t_Wn[:])
        sb_ones = cpool.tile([1, 128], FP32, tag="ones")
        nc.sync.dma_start(out=sb_ones[:], in_=t_ones[:])
        sb_gr = cpool.tile([1, OUT_DIM], FP32, tag="gr")
        nc.sync.dma_start(out=sb_gr[:], in_=t_gam[:])
        sb_br = cpool.tile([1, OUT_DIM], FP32, tag="br")
        nc.sync.dma_start(out=sb_br[:], in_=t_bet[:])
        sb_WnT = cpool.tile([16, H * IN_DIM], FP32, tag="WnT")
        for h in range(H):
            nc.sync.dma_start(out=sb_WnT[:, h * IN_DIM:(h + 1) * IN_DIM],
                              in_=t_WnT[16 * h:16 * (h + 1), :])
        sb_asT = cpool.tile([16, H], FP32, tag="asT")
        nc.sync.dma_start(out=sb_asT[:], in_=t_asT[:])
        sb_adT = cpool.tile([16, H], FP32, tag="adT")
        nc.sync.dma_start(out=sb_adT[:], in_=t_adT[:])

        # persistent SBUF state
        acc = cpool.tile([128, NWIN * 68], FP32, tag="acc")
        hown = cpool.tile([128, NWIN * 64], FP32, tag="hown")
        adst = cpool.tile([128, NWIN * 8], BF16, tag="adst")

        with tc.tile_pool(name="cps", bufs=1, space="PSUM") as cps:
            # gamma/beta broadcast rows -> [128, 64]
            ps_gb = cps.tile([128, 2 * OUT_DIM], FP32, tag="gb")
            nc.tensor.matmul(ps_gb[:, 0:OUT_DIM], sb_ones[:], sb_gr[:], start=True, stop=False)
            nc.tensor.matmul(ps_gb[:, OUT_DIM:], sb_ones[:], sb_br[:], start=False, stop=True)
            sb_gam = cpool.tile([128, OUT_DIM], FP32, tag="gamt")
            nc.vector.tensor_copy(out=sb_gam[:], in_=ps_gb[:, 0:OUT_DIM])
            sb_bet = cpool.tile([128, OUT_DIM], FP32, tag="bett")
            nc.vector.tensor_copy(out=sb_bet[:], in_=ps_gb[:, OUT_DIM:])

            # A = [A_src | A_dst]  ([64, 8]); A_src[:,h] = WnT_block_h^T @ attn_srcT[:,h]
            ps_A = cps.tile([IN_DIM, 2 * H], FP32, tag="Acomb")
            for h in range(H):
                nc.tensor.matmul(ps_A[:, h:h + 1], sb_WnT[:, h * IN_DIM:(h + 1) * IN_DIM],
                                 sb_asT[:, h:h + 1], start=True, stop=True)
                nc.tensor.matmul(ps_A[:, H + h:H + h + 1], sb_WnT[:, h * IN_DIM:(h + 1) * IN_DIM],
                                 sb_adT[:, h:h + 1], start=True, stop=True)
            # combined phase-0 rhs: [Wn | A_src] and [Wn | A_dst], bf16 [64, 68]
            rhs_a = cpool.tile([IN_DIM, 68], BF16, tag="rhsa")
            rhs_b = cpool.tile([IN_DIM, 68], BF16, tag="rhsb")
            nc.vector.tensor_copy(out=rhs_a[:, 0:64], in_=sb_Wn[:])
            nc.vector.tensor_copy(out=rhs_b[:, 0:64], in_=sb_Wn[:])
            nc.vector.tensor_copy(out=rhs_a[:, 64:68], in_=ps_A[:, 0:4])
            nc.vector.tensor_copy(out=rhs_b[:, 64:68], in_=ps_A[:, 4:8])

        for rep in range(reps):
            nc.vector.memset(acc[:], 0)
            with (
                tc.tile_pool(name=f"p0_{rep}", bufs=3) as p0,
                tc.tile_pool(name=f"p0ps_{rep}", bufs=2, space="PSUM") as p0ps,
            ):
                # ---- phase 0b: own nodes -> hown (f32, SBUF) + adst (bf16, SBUF) ----
                for off in range(0, NPC, 1024):
                    nn = min(1024, NPC - off)
                    nsl = (nn + 127) // 128
                    lhs = p0.tile([IN_DIM, 1024], BF16, tag="lhs")
                    if nn < 1024:
                        nc.vector.memset(lhs[:], 0)
                    nc.sync.dma_start(out=lhs[:, 0:nn], in_=t_xTo[:, off:off + nn])
                    for blk in range(0, nsl, 4):
                        nb_s = min(4, nsl - blk)
                        ps = p0ps.tile([128, 4, 68], FP32, tag="p0ps")
                        for i in range(nb_s):
                            s = blk + i
                            nc.tensor.matmul(ps[:, i, :], lhs[:, s * 128:(s + 1) * 128],
                                             rhs_b[:], start=True, stop=True)
                        w0 = (off // 128) + blk
                        nc.vector.tensor_copy(
                            out=hown[:, w0 * 64:(w0 + nb_s) * 64].rearrange(
                                "p (a b) -> p a b", a=nb_s),
                            in_=ps[:, 0:nb_s, 0:64])
                        adw = adst[:, w0 * 8:(w0 + nb_s) * 8].rearrange(
                            "p (a b) -> p a b", a=nb_s)
                        nc.vector.tensor_copy(out=adw[:, :, 0:4], in_=ps[:, 0:nb_s, 64:68])
                        nc.vector.tensor_tensor(out=adw[:, :, 4:8],
                                                in0=ps[:, 0:nb_s, 64:68],
                                                in1=adw[:, :, 0:4], op=OP.subtract)

                # ---- phase 0a: all N nodes -> tab chunks ----
                for c in range(NCHUNK):
                    for off in range(0, CHUNKP, 1024):
                        rows_here = min(1024, CHUNKP - off)
                        nn = min(rows_here, CHUNK - off)
                        lhs = p0.tile([IN_DIM, 1024], BF16, tag="lhs")
                        if nn < 1024:
                            nc.vector.memset(lhs[:], 0)
                        nc.sync.dma_start(out=lhs[:, 0:nn],
                                          in_=t_xT[:, c * CHUNK + off:c * CHUNK + off + nn])
                        for blk in range(0, rows_here // 128, 4):
                            ps = p0ps.tile([128, 4, 68], FP32, tag="p0ps")
                            for i in range(4):
                                s = blk + i
                                nc.tensor.matmul(ps[:, i, :], lhs[:, s * 128:(s + 1) * 128],
                                                 rhs_a[:], start=True, stop=True)
                            stg = p0.tile([128, 4, 72], I16, tag="stg")
                            nc.vector.tensor_copy(out=stg[:, :, 0:64].bitcast(BF16),
                                                  in_=ps[:, :, 0:64])
                            nc.vector.tensor_copy(out=stg[:, :, 64:72].bitcast(FP32),
                                                  in_=ps[:, :, 64:68])
                            n0 = off + blk * 128
                            nc.sync.dma_start(
                                out=t_tabs[c][n0:n0 + 512, 0:72].rearrange(
                                    "(b p) c -> p b c", p=128),
                                in_=stg[:])

            # ---- main loop ----
            with (
                tc.tile_pool(name=f"mn_{rep}", bufs=3) as mp3,
                tc.tile_pool(name=f"mn2_{rep}", bufs=2) as mp2,
                tc.tile_pool(name=f"fin_{rep}", bufs=2) as fp,
                tc.tile_pool(name=f"trps_{rep}", bufs=1, space="PSUM") as trps,
                tc.tile_pool(name=f"lgps_{rep}", bufs=2, space="PSUM") as lgps,
                tc.tile_pool(name=f"agps_{rep}", bufs=2, space="PSUM") as agps,
            ):
                t_glob = 0
                agg_ps = None
                for c in range(NCHUNK):
                    run_tiles = []
                    for wdx in range(NWIN):
                        for i in range(int(T[c, wdx])):
                            run_tiles.append((wdx, i, int(T[c, wdx])))
                    tcn = tiles_chunk[c]
                    assert tcn % GB == 0 and len(run_tiles) == tcn
                    for b in range(tcn // GB):
                        tb0 = t_glob + b * GB
                        e0 = tb0 * 128
                        meta_sb = mp3.tile([128, 72], I16, tag="meta")
                        nc.sync.dma_start(out=meta_sb[:], in_=t_meta[:, tb0 // GB * 72:tb0 // GB * 72 + 72])
                        attr_sb = mp3.tile([16, 1024], BF16, tag="attr")
                        nc.sync.dma_start(out=attr_sb[:], in_=t_attrT[:, e0:e0 + 1024])
                        xa = mp3.tile([128, GB, 72], I16, tag="xa")
                        dma_gather_small(nc.gpsimd, xa[:], t_tabs[c][:, 0:72],
                                         meta_sb[:, 0:64], 1024, 72, 128)
                        dstc_b = meta_sb[:, 64:72].bitcast(BF16)
                        ind8 = mp2.tile([128, GB, 128], BF16, tag="ind8")
                        nc.vector.tensor_tensor(
                            out=ind8[:],
                            in0=sb_iota[:].unsqueeze(1).broadcast_to([128, GB, 128]),
                            in1=dstc_b.unsqueeze(2).broadcast_to([128, GB, 128]),
                            op=OP.is_equal)
                        trans_ps = trps.tile([128, GB, 128], FP32, tag="tr")
                        for j in range(GB):
                            nc.tensor.matmul(trans_ps[:, j, :], ind8[:, j, :], sb_iden[:],
                                             start=True, stop=True)
                        indT8 = mp2.tile([128, GB, 128], BF16, tag="indT8")
                        nc.scalar.activation(
                            out=indT8[:].rearrange("p a b -> p (a b)"),
                            in_=trans_ps[:].rearrange("p a b -> p (a b)"), func=AF.Copy)
                        logit_ps = lgps.tile([128, GB, 8], FP32, tag="lg")
                        for j in range(GB):
                            wdx = run_tiles[b * GB + j][0]
                            nc.tensor.matmul(logit_ps[:, j, :],
                                             attr_sb[:, j * 128:(j + 1) * 128], sb_We8[:],
                                             start=True, stop=False)
                            nc.tensor.matmul(logit_ps[:, j, :], indT8[:, j, :],
                                             adst[:, wdx * 8:(wdx + 1) * 8],
                                             start=False, stop=True)
                        lg = mp2.tile([128, GB, 4], FP32, tag="lgs")
                        nc.vector.tensor_tensor(out=lg[:], in0=logit_ps[:, :, 0:4],
                                                in1=xa[:, :, 64:72].bitcast(FP32), op=OP.add)
                        nc.vector.tensor_tensor(out=lg[:], in0=logit_ps[:, :, 4:8],
                                                in1=lg[:], op=OP.add)
                        lg2 = lg[:].rearrange("p a b -> p (a b)")
                        e1 = mp2.tile([128, GB * 4], FP32, tag="e1")
                        nc.scalar.activation(out=e1[:], in_=lg2, func=AF.Exp)
                        w2 = mp2.tile([128, GB * 4], FP32, tag="w2")
                        nc.scalar.activation(out=w2[:], in_=lg2, func=AF.Exp, scale=k_neg)
                        msg = mp2.tile([128, GB, 68], BF16, tag="msg")
                        nc.vector.tensor_tensor(
                            out=msg[:, :, 64:68],
                            in0=e1[:].rearrange("p (a b) -> p a b", a=GB),
                            in1=w2[:].rearrange("p (a b) -> p a b", a=GB), op=OP.max)
                        nc.vector.tensor_tensor(
                            out=msg[:, :, 0:64].rearrange("p a (h d) -> p a h d", h=4),
                            in0=xa[:, :, 0:64].bitcast(BF16).rearrange("p a (h d) -> p a h d", h=4),
                            in1=msg[:, :, 64:68].unsqueeze(3).broadcast_to([128, GB, 4, 16]),
                            op=OP.mult)
                        for j in range(GB):
                            wdx, i_run, rlen = run_tiles[b * GB + j]
                            if i_run == 0:
                                agg_ps = agps.tile([128, 68], FP32, tag="agg")
                            nc.tensor.matmul(agg_ps[:], ind8[:, j, :], msg[:, j, :],
                                             start=(i_run == 0), stop=(i_run == rlen - 1))
                            if i_run == rlen - 1:
                                if c < last_c[wdx]:
                                    nc.vector.tensor_tensor(
                                        out=acc[:, wdx * 68:(wdx + 1) * 68],
                                        in0=acc[:, wdx * 68:(wdx + 1) * 68],
                                        in1=agg_ps[:], op=OP.add)
                                else:
                                    # ---- finalize window wdx ----
                                    n0 = wdx * 128
                                    nnw = min(128, NPC - n0)
                                    tot = fp.tile([128, 68], FP32, tag="tot")
                                    nc.vector.tensor_tensor(
                                        out=tot[:], in0=acc[:, wdx * 68:(wdx + 1) * 68],
                                        in1=agg_ps[:], op=OP.add)
                                    rcp = fp.tile([128, 4], FP32, tag="rcp")
                                    nc.vector.tensor_scalar(
                                        out=rcp[:], in0=tot[:, 64:68], scalar1=k_epssm,
                                        scalar2=None, op0=OP.add)
                                    nc.vector.reciprocal(out=rcp[:], in_=rcp[:])
                                    y = fp.tile([128, 64], FP32, tag="y")
                                    nc.vector.tensor_tensor(
                                        out=y[:].rearrange("p (a b) -> p a b", a=4),
                                        in0=tot[:, 0:64].rearrange("p (a b) -> p a b", a=4),
                                        in1=rcp[:].unsqueeze(2).broadcast_to([128, 4, 16]),
                                        op=OP.mult)
                                    nc.vector.tensor_tensor(
                                        out=y[:], in0=y[:],
                                        in1=hown[:, wdx * 64:(wdx + 1) * 64], op=OP.add)
                                    mus = fp.tile([128, 1], FP32, tag="mus")
                                    nc.vector.tensor_reduce(out=mus[:], in_=y[:],
                                                            axis=AX.X, op=OP.add)
                                    sq = fp.tile([128, 64], FP32, tag="sq")
                                    sqs = fp.tile([128, 1], FP32, tag="sqs")
                                    nc.scalar.activation(out=sq[:], in_=y[:],
                                                         func=AF.Square, accum_out=sqs[:])
                                    mu = fp.tile([128, 1], FP32, tag="mu")
                                    nc.vector.tensor_scalar(out=mu[:], in0=mus[:],
                                                            scalar1=1.0 / 64.0,
                                                            scalar2=None, op0=OP.mult)
                                    m2e = fp.tile([128, 1], FP32, tag="m2e")
                                    nc.vector.tensor_scalar(out=m2e[:], in0=mu[:],
                                                            scalar1=mu[:], scalar2=EPS_LN,
                                                            op0=OP.mult, op1=OP.subtract)
                                    veps = fp.tile([128, 1], FP32, tag="veps")
                                    nc.vector.tensor_scalar(out=veps[:], in0=sqs[:],
                                                            scalar1=k_inv64, scalar2=m2e[:],
                                                            op0=OP.mult, op1=OP.subtract)
                                    rstd = fp.tile([128, 1], FP32, tag="rstd")
                                    nc.scalar.activation(out=rstd[:], in_=veps[:], func=AF.Sqrt)
                                    nc.vector.reciprocal(out=rstd[:], in_=rstd[:])
                                    nmr = fp.tile([128, 1], FP32, tag="nmr")
                                    nc.vector.tensor_scalar(out=nmr[:], in0=mu[:],
                                                            scalar1=rstd[:], scalar2=-1.0,
                                                            op0=OP.mult, op1=OP.mult)
                                    z = fp.tile([128, 64], FP32, tag="z")
                                    nc.vector.tensor_scalar(out=z[:], in0=y[:],
                                                            scalar1=rstd[:], scalar2=nmr[:],
                                                            op0=OP.mult, op1=OP.add)
                                    o = fp.tile([128, 64], FP32, tag="o")
                                    nc.vector.tensor_tensor(out=o[:], in0=z[:],
                                                            in1=sb_gam[:], op=OP.mult)
                                    nc.vector.tensor_tensor(out=o[:], in0=o[:],
                                                            in1=sb_bet[:], op=OP.add)
                                    nc.sync.dma_start(out=t_out[n0:n0 + nnw, :],
                                                      in_=o[0:nnw, :])
                    t_glob += tcn

    nc.compile()
    return nc


class _Runner:
    def __init__(self, nc):
        import jax
        from jax.sharding import Mesh, PartitionSpec, NamedSharding
        from jax.experimental.shard_map import shard_map
        from concourse import bass2jax as b2j

        b2j.install_neuronx_cc_hook()
        self.jax = jax
        self.nc = nc
        pname = nc.partition_id_tensor.name if nc.partition_id_tensor else None
        in_names, out_names, out_avals, zero_outs = [], [], [], []
        for alloc in nc.m.functions[0].allocations:
            if not isinstance(alloc, mybir.MemoryLocationSet):
                continue
            name = alloc.memorylocations[0].name
            if alloc.kind == "ExternalInput":
                if name != pname:
                    in_names.append(name)
            elif alloc.kind == "ExternalOutput":
                shape = tuple(alloc.tensor_shape)
                dtype = mybir.dt.np(alloc.dtype)
                out_names.append(name)
                out_avals.append(jax.core.ShapedArray(shape, dtype))
                zero_outs.append((shape, dtype))
        self.in_names = in_names
        self.out_names = out_names
        n_params = len(in_names)
        all_in = list(in_names) + list(out_names)
        if pname is not None:
            all_in.append(pname)

        def _body(*args):
            operands = list(args)
            if pname is not None:
                operands.append(b2j.partition_id_tensor())
            return tuple(b2j._bass_exec_p.bind(
                *operands, out_avals=tuple(out_avals), in_names=tuple(all_in),
                out_names=tuple(out_names), lowering_input_output_aliases=(),
                sim_require_finite=True, sim_require_nnan=True, nc=nc))

        devices = jax.devices()[:NCORES]
        mesh = Mesh(np.asarray(devices), ("core",))
        self.sh = NamedSharding(mesh, PartitionSpec("core"))
        self.fn = jax.jit(
            shard_map(_body, mesh=mesh,
                      in_specs=(PartitionSpec("core"),) * (n_params + len(zero_outs)),
                      out_specs=(PartitionSpec("core"),) * len(out_names),
                      check_rep=False),
            donate_argnums=tuple(range(n_params, n_params + len(zero_outs))),
            keep_unused=True)
        import jax.numpy as jnp
        self.zmaker = jax.jit(
            lambda: tuple(jnp.zeros((NCORES * s[0], *s[1:]), d) for s, d in zero_outs),
            out_shardings=(self.sh,) * len(zero_outs))

    def stage(self, in_maps):
        concat = {
            n: np.concatenate([np.asarray(in_maps[c][n]) for c in range(NCORES)], axis=0)
            for n in self.in_names
        }
        staged = [self.jax.device_put(concat[n], self.sh) for n in self.in_names]
        self.jax.block_until_ready(staged)
        return staged

    def run(self, staged):
        zs = self.zmaker()
        self.jax.block_until_ready(zs)
        outs = self.fn(*staged, *zs)
        self.jax.block_until_ready(outs)
        return outs

    def timed_run(self, staged):
        import time
        zs = self.zmaker()
        self.jax.block_until_ready(zs)
        t0 = time.time()
        outs = self.fn(*staged, *zs)
        self.jax.block_until_ready(outs)
        return time.time() - t0, outs


def _get_runner(T, Etot, reps):
    key = (T.tobytes(), Etot, reps)
    if key not in _CACHE:
        nc = _build_program(T, Etot, reps)
        _CACHE[key] = _Runner(nc)
    return _CACHE[key]


def _make_in_maps(x, W_node, W_edge, attn_src, attn_dst, ln_gamma, ln_beta, streams):
    xT_bf = np.ascontiguousarray(np.asarray(x, np.float32).T).astype(ml_dtypes.bfloat16)
    shared = dict(
        xT_bf=xT_bf,
        WnT=np.ascontiguousarray(np.asarray(W_node, np.float32).T),
        attn_srcT=np.ascontiguousarray(np.asarray(attn_src, np.float32).T),
        attn_dstT=np.ascontiguousarray(np.asarray(attn_dst, np.float32).T),
        W_edge=np.asarray(W_edge, np.float32),
        W_node=np.asarray(W_node, np.float32),
        gamma_row=np.ascontiguousarray(np.asarray(ln_gamma, np.float32)[None, :]),
        beta_row=np.ascontiguousarray(np.asarray(ln_beta, np.float32)[None, :]),
        iota_bf=np.tile(np.arange(128, dtype=np.float32)[None, :], (128, 1)).astype(ml_dtypes.bfloat16),
        iden_bf=np.eye(128, dtype=np.float32).astype(ml_dtypes.bfloat16),
        ones_col=np.ones((1, 128), np.float32),
    )
    in_maps = []
    for k in range(NCORES):
        m = dict(shared)
        m["xT_own"] = np.ascontiguousarray(xT_bf[:, k * NPC:(k + 1) * NPC])
        m["meta"] = streams[k]["meta"]
        m["attrT"] = streams[k]["attrT"]
        in_maps.append(m)
    return in_maps


def kernel(x, edge_index, edge_attr, W_node, W_edge, attn_src, attn_dst, ln_gamma, ln_beta):
    global _LAST_STAGED, LAST_EXEC_NS
    import time as _time
    streams, T, Etot = _host_prep(edge_index, edge_attr)
    runner = _get_runner(T, Etot, 1)
    in_maps = _make_in_maps(x, W_node, W_edge, attn_src, attn_dst,
                            ln_gamma, ln_beta, streams)
    staged = runner.stage(in_maps)
    _LAST_STAGED = (T, Etot, staged)
    dt, outs = runner.timed_run(staged)
    LAST_EXEC_NS = int(dt * 1e9)
    oi = runner.out_names.index("out")
    out = np.asarray(outs[oi]).reshape(NCORES * NPC, OUT_DIM)
    return out.astype(np.float32)


def benchmark(reps=3, iters=6):
    """Marginal per-execution HW time via a reps-times-repeated program.

    Returns ns per kernel execution: (t_reps - t_1) / (reps - 1), medians
    over `iters` interleaved timed runs, using inputs staged by the last
    kernel() call.
    """
    assert _LAST_STAGED is not None, "call kernel() first"
    T, Etot, staged = _LAST_STAGED
    r1 = _get_runner(T, Etot, 1)
    rR = _get_runner(T, Etot, reps)
    # warmup both
    r1.timed_run(staged)
    rR.timed_run(staged)
    t1s, tRs = [], []
    for _ in range(iters):
        t1s.append(r1.timed_run(staged)[0])
        tRs.append(rR.timed_run(staged)[0])
    t1 = float(np.median(t1s))
    tR = float(np.median(tRs))
    per_med = (tR - t1) / (reps - 1)
    per_min = (min(tRs) - min(t1s)) / (reps - 1)
    per = max(per_med if per_med > 0 else per_min, 1e-9)
    return dict(t1=t1, tR=tR, reps=reps, per_exec_ns=int(per * 1e9),
                per_med_ns=int(per_med * 1e9), per_min_ns=int(per_min * 1e9),
                t1s=t1s, tRs=tRs)
